# revision 2
# baseline (speedup 1.0000x reference)
"""Trainium2 Bass kernel for nn_AttentionalGNN (gnn_message_passing), v2.

Same exact algebraic collapse as v1 (see kernel.py docstring): the 4000-edge
attention collapses to a 200-node attention with count weights; lin/q/k/v/o
fold into G (Gram), RV (per-key bias), VO (o-projected values) on the host.

v2 speedups over v1 (30852 ns):
* all matmul operands bf16 (1 cycle/col on the PE instead of fp32's 4);
  host experiment puts the end-to-end quantization error at ~3e-4 << 2e-2.
* the per-key bias rv_h.x_w folds into th via 4 rank-1 matmuls (th' = G_h x
  + rv_h 1^T), and ln(count) becomes the exp's per-partition ACT bias, so
  each chunk needs ONE big exp instead of 4 biased ones.
* v/o bias and o bias fold into a single bc (x) deg rank-1 matmul into the
  scatter PSUM -- the 768 KB boB/obB broadcast packs are gone entirely.
* softmax normalization is one broadcast tensor_tensor multiply per query
  chunk (rz broadcast over the head blocks) instead of 8 chained ops.
* packs are bf16 and ~60% smaller; DMA is split by need-time.
"""

import numpy as np

N = 200          # nodes
C = 128          # channels
H = 4            # heads
DH = 128         # head dim
QKV = 512        # H * DH
_CHUNKS = ((0, 128), (128, 72))   # (start, size) chunks of the node axis
N_CORES = 8

PACKA = N + QKV             # xT | G1
PACKB = QKV + 2 * N         # VO1 | MT0 | MT1
PACKC = QKV + QKV           # G2 | VO2
PACKROW = QKV + QKV + N + C + C   # rv1 | rv2 | deg | bc1 | bc2

_CACHE = {}
DEBUG = False


def _build_program():
    import concourse.mybir as mybir
    import concourse.tile as tile
    from concourse import bacc

    f32 = mybir.dt.float32
    bf16 = mybir.dt.bfloat16
    AF = mybir.ActivationFunctionType
    OP = mybir.AluOpType

    nc = bacc.Bacc("TRN2", target_bir_lowering=False)

    din = {}
    for name, shape, dt_ in (
        ("packA", [C, PACKA], bf16),
        ("packB", [C, PACKB], bf16),
        ("packC", [C, PACKC], bf16),
        ("packRow", [1, PACKROW], bf16),
        ("lncF", [C, 2], f32),
    ):
        din[name] = nc.dram_tensor(name, shape, dt_, kind="ExternalInput")
    y_d = nc.dram_tensor("yT", [1, C, 1, N], f32, kind="ExternalOutput")
    dbg = {}
    if DEBUG:
        for name, shape, dt_ in (
            ("d_th1", [C, 2 * 2 * N], mybir.dt.bfloat16),
            ("d_pt0", [C, 2 * 2 * N], mybir.dt.bfloat16),
            ("d_vt0", [C, H * (DH + 1)], mybir.dt.bfloat16),
            ("d_msg0", [C, 2 * 2 * DH], mybir.dt.bfloat16),
            ("d_hT", [C, N], mybir.dt.bfloat16),
            ("d_A0", [C, 2 * 2 * 129], mybir.dt.float32),
            ("d_rz0", [C, 4], mybir.dt.float32),
        ):
            dbg[name] = nc.dram_tensor(name, shape, dt_, kind="ExternalOutput")

    with tile.TileContext(nc) as tc:
        with (
            tc.tile_pool(name="singles", bufs=1) as singles,
            tc.tile_pool(name="work", bufs=2) as work,
            tc.tile_pool(name="psum", bufs=1, space="PSUM") as psum,
        ):
            # --- constants + warm-up (no DMA deps; overlap the input DMA) ---
            junk_bf = singles.tile([1, 512], bf16, tag="w_junk_bf")
            nc.gpsimd.memset(junk_bf[:, :288], 1.0)
            out_sb = singles.tile([128, N], f32, tag="w_out_sb")
            nc.vector.memset(out_sb[:], 0.0)
            zidx = singles.tile([128, 1], mybir.dt.int32, tag="w_zidx")
            nc.gpsimd.memset(zidx[:], 0)
            ysem = nc.alloc_semaphore("ysem")
            prow = singles.tile([1, PACKROW], bf16, tag="w_prow")
            nc.scalar.dma_start(prow[:], din["packRow"][:])
            ones_bf = singles.tile([1, N], bf16, tag="w_ones_bf")
            nc.gpsimd.memset(ones_bf[:], 1.0)

            # PE p-state ramp: start the PE early and keep it busy until
            # the first weight DMA lands (~3.3us); the ramp hits full speed
            # ~3us after the streak starts and survives pipeline gaps.
            # p-state ramp: junk matmuls on an uninitialized tile (the
            # results are never read) keep the PE busy from ~0.1us so the
            # ramp to full speed completes before the first real matmul
            for _ in range(9):
                ps = psum.tile([128, 512], f32, tag="b1", bufs=2)
                nc.tensor.matmul(ps[:, :288], junk_bf[:1, :128],
                                 junk_bf[:1, :288], start=True, stop=True)

            # ACT table: explicitly load func-set 6
            # (natural_log_exp_and_others) -- it holds exp, ln AND copy, so
            # the auto-inserter adds no further loads and the tail Ln does
            # not pay a 1.3us reload.
            nc.scalar.add_instruction(mybir.InstLoadActFuncSet(
                act_func_set_id=6,
                name=nc.get_next_instruction_name(),
                ins=[], outs=[]))

            # --- input DMA, split by need-time ---
            pA = singles.tile([C, PACKA], bf16, tag="w_pA")
            nc.sync.dma_start(pA[:], din["packA"][:])
            pB = singles.tile([C, PACKB], bf16, tag="w_pB")
            nc.sync.dma_start(pB[:, 0:QKV], din["packB"][:, 0:QKV])
            nc.sync.dma_start(pB[:, QKV:], din["packB"][:, QKV:])
            pC = singles.tile([C, PACKC], bf16, tag="w_pC")
            nc.sync.dma_start(pC[:], din["packC"][:])
            plnc = singles.tile([C, 2], f32, tag="w_plnc")
            nc.scalar.dma_start(plnc[:], din["lncF"][:])

            xT = pA[:, 0:N]
            W = {
                "G1": pA[:, N:N + QKV],
                "VO1": pB[:, 0:QKV],
                "G2": pC[:, 0:QKV],
                "VO2": pC[:, QKV:2 * QKV],
            }
            MT_sb = [pB[:, QKV:QKV + N], pB[:, QKV + N:QKV + 2 * N]]
            rv_row = [prow[0:1, 0:QKV], prow[0:1, QKV:2 * QKV]]
            deg_row = prow[0:1, 2 * QKV:2 * QKV + N]
            bc_row = [prow[0:1, 2 * QKV + N:2 * QKV + N + C],
                      prow[0:1, 2 * QKV + N + C:2 * QKV + N + 2 * C]]

            saved = {}

            def gnn_layer(L, x_in):
                """x_in: SBUF [C, N] bf16 feature-major. Returns the scatter
                PSUM tile whose [:, :N] holds the fp32 layer output."""
                G = W[f"G{L}"]
                VO = W[f"VO{L}"]
                rvr = rv_row[L - 1]

                # th' = G_h^T x + rv_h 1^T, one PSUM bank per head-pair
                # (separate tiles so hp1's matmuls don't serialize behind
                # the hp0 copy), copied per pair so hp0 scores start early.
                thp = [psum.tile([128, 512], f32, tag="b1", bufs=2,
                                 name=f"thp{hp}") for hp in range(2)]
                th_sb = work.tile([128, 2, 2 * N], bf16, tag="th_sb")
                for hp in range(2):
                    for hh in range(2):
                        h = hp * 2 + hh
                        nc.tensor.matmul(thp[hp][:, hh * N:(hh + 1) * N],
                                         G[:, h * C:(h + 1) * C], x_in,
                                         start=True, stop=False)
                        nc.tensor.matmul(thp[hp][:, hh * N:(hh + 1) * N],
                                         rvr[:, h * C:(h + 1) * C],
                                         ones_bf[:1, :N],
                                         start=False, stop=True)
                    eng = nc.vector.tensor_copy if hp == 0 else nc.scalar.copy
                    eng(out=th_sb[:, hp, :], in_=thp[hp][:, :2 * N])

                # scores (hp0 for both chunks first), then exp with ln(count)
                # as the per-partition bias
                sp = []
                for ci, (w0, wc) in enumerate(_CHUNKS):
                    sp.append(psum.tile([128, 2, 512], f32, tag="b2", bufs=3,
                                        name=f"sp{ci}"))
                for hp in range(2):
                    for ci, (w0, wc) in enumerate(_CHUNKS):
                        nc.tensor.matmul(sp[ci][:wc, hp, :2 * N],
                                         x_in[:, w0:w0 + wc],
                                         th_sb[:, hp, :],
                                         start=True, stop=True)
                PT = []
                for ci, (w0, wc) in enumerate(_CHUNKS):
                    pt = work.tile([128, 2, 2 * N], bf16, tag=f"PT{ci}")
                    nc.scalar.activation(out=pt[:wc], in_=sp[ci][:wc, :, :2 * N],
                                         func=AF.Exp,
                                         bias=plnc[:wc, ci:ci + 1], scale=1.0)
                    PT.append(pt)

                # vo = x^T VO (node-major) with a ones column per head (emits
                # the normalizer Z from the same attention matmul); copies on
                # DVE so the ACT queue stays free for the exps.
                vt = []
                for ci, (w0, wc) in enumerate(_CHUNKS):
                    ps = psum.tile([128, 512], f32, tag="b1", bufs=2)
                    nc.tensor.matmul(ps[:wc, :], x_in[:, w0:w0 + wc], VO[:],
                                     start=True, stop=True)
                    v = work.tile([128, H, DH + 1], bf16, tag=f"vt{ci}")
                    nc.vector.memset(v[:, :, DH:], 1.0)
                    nc.vector.tensor_copy(
                        out=v[:wc, :, :DH],
                        in_=ps[:wc, :].rearrange("p (h c) -> p h c", h=H))
                    vt.append(v)

                # scatter target: bias rank-1 opens the accumulation early
                sps = psum.tile([128, 512], f32, tag="b1", bufs=2)
                nc.tensor.matmul(sps[:, :N], bc_row[L - 1][:, :],
                                 deg_row[:, :], start=True, stop=False)

                # attention. PSUM rule: a start=True matmul clobbers any
                # OPEN accumulation group in its bank, so the two head
                # regions sharing a bank must run as consecutive closed
                # groups. The hh=0 openers still hide under exp(chunk1).
                apt = [psum.tile([128, 2, 2, 256], f32, tag="b2", bufs=3,
                                 name=f"ap{ui}") for ui in range(2)]

                def att_mm(ui, b, hh, ci):
                    u0, uc = _CHUNKS[ui]
                    w0, wc = _CHUNKS[ci]
                    nc.tensor.matmul(
                        apt[ui][:uc, b, hh, :DH + 1],
                        PT[ci][:wc, b, hh * N + u0:hh * N + u0 + uc],
                        vt[ci][:wc, 2 * b + hh, :],
                        start=(ci == 0), stop=(ci == 1))

                for ui in range(2):
                    for b in range(2):
                        att_mm(ui, b, 0, 0)       # openers: need only exp c0
                for ui in range(2):
                    for b in range(2):
                        att_mm(ui, b, 0, 1)       # close hh0 (needs exp c1)
                        att_mm(ui, b, 1, 0)       # then hh1 open+close
                        att_mm(ui, b, 1, 1)
                # normalize per ui (one reciprocal + one broadcast multiply),
                # then scatter that ui's four head blocks
                ms, rzs = [], []
                for ui in range(2):
                    rzs.append(work.tile([128, 2, 2, 1], f32, tag=f"rz{ui}",
                                         name=f"rz{ui}"))
                    ms.append(work.tile([128, 2, 2, DH], bf16, tag=f"msg{ui}",
                                        name=f"msg{ui}"))
                saved["_rz0"] = rzs[0]
                nc.vector.reciprocal(out=rzs[0][:128],
                                     in_=apt[0][:128, :, :, DH:DH + 1])
                for b in range(2):      # ui0 normalized per head-pair so the
                    nc.vector.tensor_mul(   # scatter can start sooner
                        ms[0][:128, b], apt[0][:128, b, :, :DH],
                        rzs[0][:128, b].broadcast_to([128, 2, DH]))
                    for hh in range(2):
                        nc.tensor.matmul(sps[:, :N], ms[0][:128, b, hh, :],
                                         MT_sb[0][:128, :],
                                         start=False, stop=False)
                uc1 = _CHUNKS[1][1]
                nc.vector.reciprocal(out=rzs[1][:uc1],
                                     in_=apt[1][:uc1, :, :, DH:DH + 1])
                nc.vector.tensor_mul(
                    ms[1][:uc1], apt[1][:uc1, :, :, :DH],
                    rzs[1][:uc1].broadcast_to([uc1, 2, 2, DH]))
                for b in range(2):
                    for hh in range(2):
                        nc.tensor.matmul(sps[:, :N], ms[1][:uc1, b, hh, :],
                                         MT_sb[1][:uc1, :],
                                         start=False,
                                         stop=(b == 1 and hh == 1))
                if DEBUG and L == 1:
                    acp = work.tile([128, 2, 2, 129], f32, tag="acp")
                    nc.vector.tensor_copy(out=acp[:], in_=apt[0][:, :, :, :129])
                    saved["acp"] = acp
                saved[L] = dict(th_sb=th_sb, PT0=PT[0], vt0=vt[0],
                                msg0=ms[0], msg1=ms[1], rz0=saved.get("_rz0"))
                return sps

            ps1 = gnn_layer(1, xT)
            if DEBUG:
                s1 = saved[1]
                nc.sync.dma_start(dbg["d_th1"][:],
                                  s1["th_sb"][:, :, :].rearrange("p a b -> p (a b)"))
                nc.sync.dma_start(dbg["d_pt0"][:],
                                  s1["PT0"][:, :, :].rearrange("p a b -> p (a b)"))
                nc.sync.dma_start(dbg["d_vt0"][:],
                                  s1["vt0"][:, :, :].rearrange("p a b -> p (a b)"))
                nc.sync.dma_start(dbg["d_msg0"][:, :256],
                                  s1["msg0"][:, 0].rearrange("p a b -> p (a b)"))
                nc.sync.dma_start(dbg["d_msg0"][:, 256:],
                                  s1["msg0"][:, 1].rearrange("p a b -> p (a b)"))
                nc.sync.dma_start(dbg["d_A0"][:],
                                  saved["acp"][:, :, :, :].rearrange("p a b c -> p (a b c)"))
                nc.sync.dma_start(dbg["d_rz0"][:],
                                  s1["rz0"][:, :, :, 0].rearrange("p a b -> p (a b)"))
            hT = work.tile([C, N], bf16, tag="hT")
            nc.vector.tensor_scalar_max(out=hT[:], in0=ps1[:, :N], scalar1=0.0)
            if DEBUG:
                nc.sync.dma_start(dbg["d_hT"][:], hT[:])
            ps2 = gnn_layer(2, hT)

            # log_softmax over the node axis (free dim); |pre-softmax| is
            # bounded (~25) so exp is safe without max-subtraction.
            esum = work.tile([128, 1], f32, tag="esum")
            etmp = work.tile([128, N], bf16, tag="etmp")
            nc.scalar.activation(out=etmp[:], in_=ps2[:, :N], func=AF.Exp,
                                 scale=1.0, accum_out=esum[:])
            lse = work.tile([128, 1], f32, tag="lse")
            nc.scalar.activation(out=lse[:], in_=esum[:], func=AF.Ln)
            nc.vector.tensor_scalar_sub(out=out_sb[:], in0=ps2[:, :N],
                                        scalar1=lse[:])
            # writeback: desc-gen after the sub (the scheduler will not
            # honor an earlier prep's deferred data edge on real HW), then
            # trigger fires the transfer
            nc.gpsimd.kv_writeback(
                y_d[:], out_sb[:].rearrange("p (a b n) -> p a b n", a=1, b=1),
                zidx[:], prepare_only=True, sem=ysem)
            nc.gpsimd.trigger_dma(count=None)
            nc.gpsimd.wait_ge(ysem, 16)

    nc.compile()
    # The kv_writeback prep is tracked on the framework's DMASW0 lane, but its
    # completion semaphore is ysem (baked into the descriptor), so the
    # epilogue's DMASW0 wait would never be satisfied. The explicit
    # wait_ge(ysem) on the Pool queue already orders the barrier after the
    # writeback, so drop the stale DMASW waits.
    for blk in nc.m.functions[0].blocks:
        for inst in blk.instructions:
            si = inst.sync_info
            if si is None or not isinstance(inst, mybir.InstEventSemaphore):
                continue
            waits = list(si.on_wait or [])
            keep = [w for w in waits
                    if not (w.ant_name or "").startswith("DMASW")]
            if len(keep) != len(waits):
                inst.sync_info = mybir.SyncInfo(
                    on_wait=keep, on_update=list(si.on_update or []))
    return nc


def _prep_inputs(x, edge_index, params):
    """Host-side preprocessing: index collapse + weight folding (float64)."""
    import ml_dtypes
    bf = ml_dtypes.bfloat16
    row = np.asarray(edge_index[0]).astype(np.int64)
    col = np.asarray(edge_index[1]).astype(np.int64)
    a = row[row]
    b = row[col]
    cb = np.bincount(b, minlength=N).astype(np.float64)
    lnc = np.where(cb > 0, np.log(np.maximum(cb, 1e-300)), -30000.0)
    M = np.zeros((N, N), np.float64)
    np.add.at(M, (col, a), 1.0)
    deg = np.bincount(col, minlength=N).astype(np.float64)

    folded = {}
    scale = np.float64(1.0) / np.sqrt(np.float64(C))
    for L in (1, 2):
        p = {k: np.asarray(params[f"l{L}_{k}"]).astype(np.float64)
             for k in ("lin_w", "lin_b", "q_w", "q_b", "k_w", "k_b",
                       "v_w", "v_b", "o_w", "o_b")}
        sqlw = (p["q_w"] @ p["lin_w"]) * scale           # [512, 128]
        sqlb = (p["q_w"] @ p["lin_b"] + p["q_b"]) * scale
        klw = p["k_w"] @ p["lin_w"]
        vlw = p["v_w"] @ p["lin_w"]
        vlb = p["v_w"] @ p["lin_b"] + p["v_b"]
        G = np.empty((C, H * C))
        RV = np.empty(H * C)
        VO = np.empty((C, QKV))
        bo = np.empty(QKV)
        for h in range(H):
            sl = slice(h * DH, (h + 1) * DH)
            G[:, sl] = sqlw[sl].T @ klw[sl]
            RV[sl] = klw[sl].T @ sqlb[sl]
            ow_h = p["o_w"][:, sl]                       # [C, DH]
            VO[:, sl] = vlw[sl].T @ ow_h.T
            bo[sl] = ow_h @ vlb[sl]
        folded[f"G{L}"] = G
        folded[f"RV{L}"] = RV
        folded[f"VO{L}"] = VO
        folded[f"bc{L}"] = bo.reshape(H, DH).sum(0) + p["o_b"]

    xT = np.asarray(x, np.float64)[0].T
    MT = M.T                                            # [u, n]
    MT0 = MT[0:128, :]
    MT1 = np.zeros((C, N))
    MT1[0:72, :] = MT[128:200, :]
    packA = np.concatenate([xT, folded["G1"]], axis=1)
    packB = np.concatenate([folded["VO1"], MT0, MT1], axis=1)
    packC = np.concatenate([folded["G2"], folded["VO2"]], axis=1)
    packRow = np.concatenate(
        [folded["RV1"], folded["RV2"], deg, folded["bc1"], folded["bc2"]]
    )[None, :]
    lncF = np.zeros((C, 2))
    lncF[0:128, 0] = lnc[0:128]
    lncF[0:72, 1] = lnc[128:200]
    assert packA.shape == (C, PACKA) and packB.shape == (C, PACKB)
    assert packC.shape == (C, PACKC) and packRow.shape == (1, PACKROW)
    return {
        "packA": np.ascontiguousarray(packA.astype(bf)),
        "packB": np.ascontiguousarray(packB.astype(bf)),
        "packC": np.ascontiguousarray(packC.astype(bf)),
        "packRow": np.ascontiguousarray(packRow.astype(bf)),
        "lncF": np.ascontiguousarray(lncF.astype(np.float32)),
    }


def run_on_device(in_map, trace=False, **kwargs):
    from concourse.bass_utils import run_bass_kernel_spmd

    if "nc" not in _CACHE:
        _CACHE["nc"] = _build_program()
    nc = _CACHE["nc"]
    res = run_bass_kernel_spmd(nc, [in_map] * N_CORES,
                               core_ids=list(range(N_CORES)),
                               trace=trace, **kwargs)
    return res


def kernel(x, edge_index, **params):
    in_map = _prep_inputs(x, edge_index, params)
    res = run_on_device(in_map)
    yT = np.asarray(res.results[0]["yT"]).reshape(C, N)
    return np.ascontiguousarray(yT.T)[None].astype(np.float32)
